# revision 3
# baseline (speedup 1.0000x reference)
"""Distributed brute-force kNN retrieval (cosine similarity) on 8 Trainium2 cores.

v4 — DMA-stream-optimized fp8 candidate-generation kernel.

Strategy:
  - Shard the feature bank along N across 8 cores (62500 rows each).
  - Host packs each shard fp8, DMA-stream-major: per partition p, the byte
    stream is ordered (group g, k-chunk k, row x), so every DMA group is one
    [128, 6*rows] transfer with up to 24 KB contiguous per partition line
    (~128 big descriptors per group). Measured ~390-423 GB/s per core with
    all 8 cores streaming concurrently.
  - Row blocks are processed as PAIRS via PE column tiling: half A lands on
    PSUM partitions 0-63 (queries x rowsA), half B on 64-127
    (tile_position=(0,64)). fp8 matmuls, fp32 PSUM accumulation.
  - Pairs taper at the end (400/400/350/100 cols) and the final DMA group is
    only 100 rows (77 KB), so the post-stream serial chain
    (MM -> ACT copy -> MAX8 -> FIND_INDEX8 -> out-DMA) runs on tiny blocks.
  - ACT copies each PSUM block to bf16 SBUF (frees the PSUM bank fast);
    DVE Max8/MaxIndex extract per-query top-8 of each pair-half block.
  - Early pairs' results are DMA'd out mid-stream; only the last pairs ride
    the tail.
  - Host maps candidates to global rows, prefilters by device value,
    rescores exactly in fp32 (normalized cosine), final top-k + gather.

Accuracy margin: top-8 of every <=500-feature block-half when only the
global top-5 is needed; fp8 dot-noise sigma (~0.7 raw-dot units) plus bf16
sim rounding is tiny vs the many-sigma gaps among 500-sample extremes. The
exact fp32 host rescore removes all remaining matmul error.
"""

import os
import sys

import numpy as np

import concourse.bacc as bacc
import concourse.mybir as mybir
from concourse.tile import TileContext
from concourse.bass_utils import run_bass_kernel_spmd


def _ensure_ntff_hook():
    """run_bass_kernel_spmd(trace) under axon imports antenv.axon_hooks,
    which this container image lacks. Provide the shim (profiling works) or
    disable tracing so a stray BASS_TRACE env var cannot crash the run."""
    try:
        import antenv.axon_hooks  # noqa: F401
        return
    except ImportError:
        pass
    try:
        import types
        from trn_agent_boot.trn_boot import _ntff_profile_via_ctypes
        hook = _ntff_profile_via_ctypes("/opt/axon/libaxon_pjrt.so")
        mod = types.ModuleType("antenv.axon_hooks")
        mod.get_axon_ntff_profile_hook = lambda: hook
        mod.set_axon_ntff_profile_hook = lambda h: None
        sys.modules["antenv.axon_hooks"] = mod
        import antenv
        antenv.axon_hooks = mod
    except Exception:
        os.environ["BASS_NEVER_TRACE"] = "1"

# Problem geometry (hardcoded per spec).
B = 64             # queries
D = 768            # feature dim
N = 500000         # feature rows
NCORES = 8
NSH = N // NCORES  # 62500 rows per core
KC = D // 128      # 6 contraction chunks of 128
TOPB = 8

# Row-range carve: 60 full pairs of (500, 500), then tapered pairs so the
# post-stream tail operates on shrinking blocks.
PAIRS = [((1000 * j, 500), (1000 * j + 500, 500)) for j in range(60)]
PAIRS += [
    ((60000, 400), (60400, 400)),
    ((60800, 400), (61200, 400)),
    ((61600, 350), (61950, 350)),
    ((62300, 100), (62400, 100)),
]
NPAIRS = len(PAIRS)  # 64
assert PAIRS[-1][1][0] + PAIRS[-1][1][1] == NSH

# DMA groups (row_start, nrows): 15 big groups, then a taper whose last
# group is only the final pair's B half (77 KB).
GROUPS = [(4000 * g, 4000) for g in range(15)]
GROUPS += [(60000, 800), (60800, 800), (61600, 800), (62400, 100)]
assert GROUPS[-1][0] + GROUPS[-1][1] == NSH
MAXG = max(nr for _, nr in GROUPS)
FBUFS = 6                        # group buffers in flight
EARLY = 56                       # pairs DMA'd out mid-stream; rest at the end
MAXCOLS = 500

_COMPILED = None
LAST_RESULTS = None  # test harness introspection


def _build():
    nc = bacc.Bacc(
        "TRN2", target_bir_lowering=False, debug=False, enable_partition_id=False
    )
    qT = nc.declare_dram_parameter("qT", [D, B], mybir.dt.float8e4, isOutput=False)
    fT = nc.declare_dram_parameter("fT", [128, KC * NSH], mybir.dt.float8e4, isOutput=False)
    out_vals = nc.declare_dram_parameter(
        "vals", [128, NPAIRS * TOPB], mybir.dt.bfloat16, isOutput=True
    )
    out_idx = nc.declare_dram_parameter(
        "idx", [128, NPAIRS * TOPB], mybir.dt.uint16, isOutput=True
    )

    qT_r = qT.ap().rearrange("(k p) m -> p k m", p=128)
    fT_ap = fT.ap()

    g_off = [0]  # per-group byte offset within a partition line
    for _, nr in GROUPS:
        g_off.append(g_off[-1] + KC * nr)

    with TileContext(nc) as tc:
        with (
            tc.tile_pool(name="qpool", bufs=1) as qpool,
            tc.tile_pool(name="fpool", bufs=FBUFS) as fpool,
            tc.tile_pool(name="simspool", bufs=4) as simspool,
            tc.tile_pool(name="outpool", bufs=1) as outpool,
            tc.tile_pool(name="psum", bufs=8, space="PSUM") as psump,
        ):
            q_sb = qpool.tile([128, KC, B], mybir.dt.float8e4)
            nc.sync.dma_start(out=q_sb[:], in_=qT_r)

            vals_st = outpool.tile([128, NPAIRS * TOPB], mybir.dt.bfloat16)
            idx_st = outpool.tile([128, NPAIRS * TOPB], mybir.dt.uint16)

            g_tiles = []   # (tile, row_start, nrows)
            loaded = [0]   # groups issued so far

            def load_group():
                g = loaded[0]
                r0, nr = GROUPS[g]
                f_sb = fpool.tile([128, KC * MAXG], mybir.dt.float8e4)
                nc.sync.dma_start(
                    out=f_sb[:, : KC * nr],
                    in_=fT_ap[:, g_off[g] : g_off[g + 1]],
                )
                g_tiles.append((f_sb, r0, nr))
                loaded[0] = g + 1

            def loaded_rows():
                if loaded[0] == 0:
                    return 0
                r0, nr = GROUPS[loaded[0] - 1]
                return r0 + nr

            def load_until(row_end):
                while loaded[0] < len(GROUPS) and loaded_rows() < row_end:
                    load_group()

            def rhs(r0, cols, k):
                for f_sb, gr0, gnr in g_tiles:
                    if gr0 <= r0 and r0 + cols <= gr0 + gnr:
                        base = k * gnr + (r0 - gr0)
                        return f_sb[:, base : base + cols]
                raise AssertionError(f"rows [{r0},{r0+cols}) not loaded")

            def mm_half(ps, r0, cols, half):
                for k in range(KC):
                    nc.tensor.matmul(
                        ps[half * B:(half + 1) * B, :cols],
                        lhsT=q_sb[:, k, :],
                        rhs=rhs(r0, cols, k),
                        start=(k == 0),
                        stop=(k == KC - 1),
                        tile_position=(0, half * B) if half else None,
                    )

            # Deep prefetch: fill all group buffers before compute starts.
            for _ in range(min(FBUFS, len(GROUPS))):
                load_group()

            for j in range(NPAIRS):
                (ra, ca), (rb, cb) = PAIRS[j]
                assert ca == cb
                load_until(rb + cb)
                cols = ca
                ps = psump.tile([128, MAXCOLS], mybir.dt.float32)
                mm_half(ps, ra, cols, 0)
                mm_half(ps, rb, cols, 1)
                sims = simspool.tile([128, MAXCOLS], mybir.dt.bfloat16)
                nc.scalar.copy(out=sims[:, :cols], in_=ps[:, :cols])
                nc.vector.max(
                    out=vals_st[:, j * TOPB:(j + 1) * TOPB], in_=sims[:, :cols]
                )
                nc.vector.max_index(
                    out=idx_st[:, j * TOPB:(j + 1) * TOPB],
                    in_max=vals_st[:, j * TOPB:(j + 1) * TOPB],
                    in_values=sims[:, :cols],
                )
                if j == EARLY - 1:
                    nc.sync.dma_start(
                        out=out_vals.ap()[:, : EARLY * TOPB],
                        in_=vals_st[:, : EARLY * TOPB],
                    )
                    nc.sync.dma_start(
                        out=out_idx.ap()[:, : EARLY * TOPB],
                        in_=idx_st[:, : EARLY * TOPB],
                    )

            nc.sync.dma_start(
                out=out_vals.ap()[:, EARLY * TOPB:], in_=vals_st[:, EARLY * TOPB:]
            )
            nc.sync.dma_start(
                out=out_idx.ap()[:, EARLY * TOPB:], in_=idx_st[:, EARLY * TOPB:]
            )

    nc.compile()
    return nc


def _get_compiled():
    global _COMPILED
    if _COMPILED is None:
        _COMPILED = _build()
    return _COMPILED


def _pack_shard(f_shard_f8):
    """[NSH, D] fp8 -> [128, KC*NSH] stream-major: per partition p, per DMA
    group, bytes ordered (k, row): A[p, goff + k*nrows + x] = f[r0 + x, k*128 + p]."""
    parts = []
    for r0, nr in GROUPS:
        sub = f_shard_f8[r0 : r0 + nr]                             # [nr, 768]
        sub = sub.T.reshape(KC, 128, nr).transpose(1, 0, 2)        # [128, KC, nr]
        parts.append(sub.reshape(128, KC * nr))
    return np.ascontiguousarray(np.concatenate(parts, axis=1))


def _candidates(idx_arr, val_arr):
    """Map device outputs (128, NPAIRS*8) to (feature rows, dot vals).

    Row p < 64 is query p over each pair's first half; row p >= 64 is query
    p-64 over the second half. Index i within pair j's half is the offset
    from that half's start row.
    """
    startA = np.repeat(np.array([p[0][0] for p in PAIRS]), TOPB)  # (NPAIRS*8,)
    startB = np.repeat(np.array([p[1][0] for p in PAIRS]), TOPB)
    rows_out, vals_out = [], []
    for half, start in ((0, startA), (1, startB)):
        i = idx_arr[half * B:(half + 1) * B].astype(np.int64)  # (B, NPAIRS*8)
        rows_out.append(start[None, :] + i)
        vals_out.append(val_arr[half * B:(half + 1) * B])
    return (
        np.concatenate(rows_out, axis=1),   # (B, 2*NPAIRS*8)
        np.concatenate(vals_out, axis=1),
    )


def kernel(query_feature, feature, data, k=5, **kwargs):
    global LAST_RESULTS
    q = np.ascontiguousarray(np.asarray(query_feature, dtype=np.float32))
    f = np.asarray(feature, dtype=np.float32)
    data = np.asarray(data)
    k = int(k)
    assert q.shape == (B, D) and f.shape == (N, D)

    nc = _get_compiled()

    F8 = mybir.dt.np(mybir.dt.float8e4)
    qT = np.ascontiguousarray(q.T.astype(F8))
    in_maps = []
    for i in range(NCORES):
        fT = _pack_shard(f[i * NSH:(i + 1) * NSH].astype(F8))
        in_maps.append({"qT": qT, "fT": fT})

    _ensure_ntff_hook()
    res = run_bass_kernel_spmd(nc, in_maps, core_ids=list(range(NCORES)))
    LAST_RESULTS = res

    all_rows, all_vals = [], []
    for i in range(NCORES):
        rows, vals = _candidates(
            res.results[i]["idx"], res.results[i]["vals"].astype(np.float32)
        )
        all_rows.append(i * NSH + rows)
        all_vals.append(vals)
    cand_all = np.concatenate(all_rows, axis=1)  # (B, NCORES*2*NPAIRS*8)
    vals_all = np.concatenate(all_vals, axis=1)

    # Prefilter by device dot value, then rescore those exactly.
    PREK = 96
    pre = np.argpartition(-vals_all, PREK, axis=1)[:, :PREK]
    cand = np.take_along_axis(cand_all, pre, axis=1)  # (B, PREK)

    # Exact fp32 rescore of candidates (same math as the reference).
    qn = q / np.linalg.norm(q, axis=1, keepdims=True)
    fc = f[cand]  # (B, C, D)
    fn = fc / np.linalg.norm(fc, axis=2, keepdims=True)
    sims = np.einsum("bd,bcd->bc", qn, fn)  # fp32

    # Final top-k with jax.lax.top_k tie-breaking (value desc, index asc).
    # Sort by index, mask duplicate neighbors (robustness; pairs are disjoint).
    o = np.argsort(cand, axis=1, kind="stable")
    cand_s = np.take_along_axis(cand, o, axis=1)
    sims_s = np.take_along_axis(sims, o, axis=1)
    dup = np.zeros_like(sims_s, dtype=bool)
    dup[:, 1:] = cand_s[:, 1:] == cand_s[:, :-1]
    sims_s = np.where(dup, -np.inf, sims_s)
    sel = np.argsort(-sims_s, axis=1, kind="stable")[:, :k]
    top_idx = np.take_along_axis(cand_s, sel, axis=1)  # (B, k)

    return data[top_idx]  # (B, k, data_cols), input dtype preserved
